# revision 16
# baseline (speedup 1.0000x reference)
"""Cross-attention kernel for 8 Trainium2 NeuronCores.

Contract: kernel(**inputs) takes FULL unsharded numpy inputs
(x [4,2048,1024], context [4,2048,1024], Wq [1024,1024], Wkv [1024,2048])
and returns the full output [4, 2048, 1024] (float32).

Sharding (hardcoded): core = b * 2 + hg handles batch b (0..3) and head
group hg (0..1) = heads hg*8 .. hg*8+7 (16 heads total, d=64). Data +
tensor parallel: no cross-core communication (softmax is per-row).

Matmuls run in bf16 (fp32 is 2-pass LOW_HIGH on the PE = half
throughput); accumulation is fp32 in PSUM. Inputs are cast to bf16 on
the host. Output is fp32.

Per-core dataflow:
  cT = context[b].T               (PE transpose, bf16)
  KT = Wk_slice.T @ cT            [512 c, 2048 j] bf16
  V  = cT.T @ Wv_slice            [2048 j, 8 h, 65] bf16 (col 64 = 1.0)
  xT = x[b].T ; QT = Wq_slice.T @ xT   [512 c, 2048 i] bf16
  per (head h, i-macro of 1024):
    for j-chunk of 128:
      S^T = K_h^T' Q_h^T          [128 j, 1024 i] PSUM f32 (K=64 matmul)
      P^T = exp(S^T / 8)          ACT, PSUM -> SBUF bf16 (no max-sub:
                                   scores ~ N(0,1), exp is range-safe)
      per i-chunk of 128 (8):     natural-form attention accumulate
        at[:, ic] += P^T[:, ic].T @ [V_h|1]    [128 i, 65] PSUM f32
                                   (8 accumulators packed into 2 banks;
                                    start=True clears a whole bank, so
                                    only the first group per bank sets it)
    out_sb[:, h*64:+64] = at[..:64] * recip(at[.., 64])   (DVE, per ic)
  DMA out_sb -> out[2048, 512] f32 DRAM (host scatters into full out)

The attention inner loop is gated by ScalarE (exp); to keep the PE's
HAM governor warm (K=8), half the xT transposes, KT[1..3], and all QT
projection chunks are emitted as a metered filler queue between heads,
giving the scheduler dependency-free PE work for every bubble.
"""

import sys

if "/opt/trn_rl_repo" not in sys.path:
    sys.path.insert(0, "/opt/trn_rl_repo")

from contextlib import ExitStack

import ml_dtypes
import numpy as np

import concourse.bass as bass  # noqa: F401  (registers AP machinery)
import concourse.mybir as mybir
from concourse import bacc
from concourse.bass_utils import run_bass_kernel_spmd
from concourse.masks import make_identity
from concourse.tile import TileContext

FP = mybir.dt.float32
BF = mybir.dt.bfloat16
P = 128
SEQ = 2048
DIM = 1024
CC = 512  # per-core channel cols (8 heads x 64)
NH = 8  # heads per core
DH = 64  # head dim
NI = SEQ // P  # 16 seq chunks
NK = DIM // P  # 8 contraction chunks
IM = 1024  # i-macro width for attention
NIM = SEQ // IM  # 2
NIC = IM // P  # 8 i-chunks per macro
SCALE = DH ** -0.5

EXP = mybir.ActivationFunctionType.Exp

_NC = None


def _build_body(nc, tc, x_d, c_d, wq_d, wk_d, wv_d, out_d):
    with ExitStack() as ctx:
        const = ctx.enter_context(tc.tile_pool(name="const", bufs=1))
        ident = const.tile([P, P], BF, name="ident")
        make_identity(nc, ident)

        ctp = ctx.enter_context(tc.tile_pool(name="ctp", bufs=1))
        xtp = ctx.enter_context(tc.tile_pool(name="xtp", bufs=1))
        ktp = ctx.enter_context(tc.tile_pool(name="ktp", bufs=4))
        qtp = ctx.enter_context(tc.tile_pool(name="qtp", bufs=4))
        vp = ctx.enter_context(tc.tile_pool(name="vp", bufs=NI))
        wp = ctx.enter_context(tc.tile_pool(name="wp", bufs=24))
        natp = ctx.enter_context(tc.tile_pool(name="natp", bufs=4))
        ptp = ctx.enter_context(tc.tile_pool(name="ptp", bufs=6))
        outp = ctx.enter_context(tc.tile_pool(name="outp", bufs=10))
        recp = ctx.enter_context(tc.tile_pool(name="recp", bufs=8))
        # PSUM budget (8 banks): sp 2x2 + at 1x2 + fill 2x1 = 8
        fillp = ctx.enter_context(tc.tile_pool(name="fillp", bufs=2, space="PSUM"))
        spsum = ctx.enter_context(tc.tile_pool(name="spsum", bufs=2, space="PSUM"))
        apsum = ctx.enter_context(tc.tile_pool(name="apsum", bufs=1, space="PSUM"))

        KT = [ktp.tile([P, SEQ], BF, name=f"kt{m}", tag="kt") for m in range(4)]
        QT = [qtp.tile([P, SEQ], BF, name=f"qt{m}", tag="qt") for m in range(4)]
        V = [vp.tile([P, NH, DH + 1], BF, name=f"v{j}", tag="v") for j in range(NI)]
        # consolidated transposed activations: [:, k, :] is the k-th
        # 128-row contraction slice (lets 4 transposes share one eviction)
        cT = ctp.tile([P, NK, SEQ], BF, name="ct", tag="act")
        xT = xtp.tile([P, NK, SEQ], BF, name="xt", tag="act2")

        def transpose_chunk(dst, src_d, i):
            # one [128, 1024] row block of src -> dst[:, :, i*128:+128];
            # 4 transposes share a PSUM bank (only the first may set
            # start: start=True clears the whole bank) and one eviction.
            nat = natp.tile([P, DIM], BF, name="nat", tag="nat")
            nc.sync.dma_start(out=nat, in_=src_d[i * P:(i + 1) * P, :])
            for half in range(2):
                tp = fillp.tile([P, 512], BF, name="tp", tag="fp")
                for q in range(4):
                    k = half * 4 + q
                    nc.tensor.matmul(
                        tp[:, q * P:(q + 1) * P],
                        nat[:, k * P:(k + 1) * P],
                        ident,
                        is_transpose=True,
                        start=(q == 0),
                        stop=(q == 3),
                        skip_group_check=True,
                    )
                nc.vector.tensor_copy(
                    dst[:, half * 4:half * 4 + 4, i * P:(i + 1) * P],
                    tp.rearrange("p (k c) -> p k c", k=4),
                )

        def proj_chunk(dst, w, src, m, i4):
            # dst[m][:, i4*512:+512] = sum_k w[k][:, m-slice].T @ src[:, k, i4]
            ps = fillp.tile([P, 512], FP, name="ps", tag="fp")
            for k in range(NK):
                nc.tensor.matmul(
                    ps,
                    w[k][:, m * P:(m + 1) * P],
                    src[:, k, i4 * 512:(i4 + 1) * 512],
                    start=(k == 0),
                    stop=(k == NK - 1),
                )
            nc.vector.tensor_copy(dst[m][:, i4 * 512:(i4 + 1) * 512], ps)

        def v_chunk(j):
            ps = fillp.tile([P, 512], FP, name="psv", tag="fp")
            for k in range(NK):
                nc.tensor.matmul(
                    ps,
                    cT[:, k, j * P:(j + 1) * P],
                    wv[k],
                    start=(k == 0),
                    stop=(k == NK - 1),
                )
            nc.vector.tensor_copy(
                V[j][:, :, 0:DH], ps.rearrange("p (h d) -> p h d", h=NH)
            )
            nc.vector.memset(V[j][:, :, DH:DH + 1], 1.0)

        # ---- minimal serial prefix ----
        for i in range(4):
            transpose_chunk(cT, c_d, i)
        wk = [wp.tile([P, CC], BF, name=f"wk{k}", tag="w") for k in range(NK)]
        wv = [wp.tile([P, CC], BF, name=f"wv{k}", tag="w") for k in range(NK)]
        wq = [wp.tile([P, CC], BF, name=f"wq{k}", tag="w") for k in range(NK)]
        for k in range(NK):
            nc.sync.dma_start(out=wk[k], in_=wk_d[k * P:(k + 1) * P, :])
            nc.sync.dma_start(out=wv[k], in_=wv_d[k * P:(k + 1) * P, :])
            nc.sync.dma_start(out=wq[k], in_=wq_d[k * P:(k + 1) * P, :])
        proj_chunk(KT, wk, cT, 0, 0)
        for j in range(4):
            v_chunk(j)
        for i in range(NIC):
            transpose_chunk(xT, x_d, i)
        proj_chunk(QT, wq, xT, 0, 0)
        proj_chunk(QT, wq, xT, 0, 1)

        # ---- j-granular filler: everything else streams through the
        # attention phase so the PE never drains (deadlines honored).
        def ct_u(i):
            return lambda: transpose_chunk(cT, c_d, i)

        def xt_u(i):
            return lambda: transpose_chunk(xT, x_d, i)

        def kt_u(m, i4):
            return lambda: proj_chunk(KT, wk, cT, m, i4)

        def qt_u(m, i4):
            return lambda: proj_chunk(QT, wq, xT, m, i4)

        def v_u(j):
            return lambda: v_chunk(j)

        filler = {
            (0, 0, 0): [ct_u(4), ct_u(5)],
            (0, 0, 1): [ct_u(6), ct_u(7)],
            (0, 0, 2): [kt_u(0, 1), v_u(4)],
            (0, 0, 3): [ct_u(8), v_u(5)],
            (0, 0, 4): [ct_u(9), v_u(6)],
            (0, 0, 5): [ct_u(10), v_u(7)],
            (0, 0, 6): [ct_u(11), kt_u(0, 2), v_u(8)],
            (0, 0, 7): [ct_u(12), v_u(9)],
            (0, 0, 8): [ct_u(13), v_u(10)],
            (0, 0, 9): [ct_u(14), v_u(11)],
            (0, 0, 10): [ct_u(15), kt_u(0, 3), v_u(12)],
            (0, 0, 11): [v_u(13)],
            (0, 0, 12): [v_u(14)],
            (0, 0, 13): [v_u(15)],
            (0, 1, 0): [kt_u(1, 0)], (0, 1, 2): [kt_u(1, 1)],
            (0, 1, 4): [kt_u(1, 2)], (0, 1, 6): [kt_u(1, 3)],
            (0, 1, 8): [qt_u(1, 0)], (0, 1, 11): [qt_u(1, 1)],
            (0, 2, 0): [kt_u(2, 0)], (0, 2, 4): [kt_u(2, 1)],
            (0, 2, 8): [kt_u(2, 2)], (0, 2, 12): [kt_u(2, 3)],
            (0, 3, 0): [qt_u(2, 0)], (0, 3, 8): [qt_u(2, 1)],
            (0, 4, 0): [kt_u(3, 0)], (0, 4, 4): [kt_u(3, 1)],
            (0, 4, 8): [kt_u(3, 2)], (0, 4, 12): [kt_u(3, 3)],
            (0, 5, 0): [qt_u(3, 0)], (0, 5, 8): [qt_u(3, 1)],
            (0, 6, 0): [xt_u(8)], (0, 6, 2): [xt_u(9)],
            (0, 6, 4): [xt_u(10)], (0, 6, 6): [xt_u(11)],
            (0, 6, 8): [xt_u(12)], (0, 6, 10): [xt_u(13)],
            (0, 6, 12): [xt_u(14)], (0, 6, 14): [xt_u(15)],
            (0, 7, 0): [qt_u(0, 2)], (0, 7, 8): [qt_u(0, 3)],
            (1, 0, 0): [qt_u(1, 2)], (1, 0, 8): [qt_u(1, 3)],
            (1, 2, 0): [qt_u(2, 2)], (1, 2, 8): [qt_u(2, 3)],
            (1, 4, 0): [qt_u(3, 2)], (1, 4, 8): [qt_u(3, 3)],
        }

        # ---------------- attention ----------------
        for imac in range(NIM):
            outs = [
                outp.tile([P, CC], FP, name=f"o{imac}_{b}", tag="o")
                for b in range(NIC)
            ]
            for h in range(NH):
                m = h // 2
                kt = KT[m]
                qt = QT[m]
                po = (h % 2) * DH
                at = apsum.tile([P, 2, 512], FP, name="at", tag="at")
                for j in range(NI):
                    units = filler.get((imac, h, j), ())
                    for thunk in units:
                        thunk()
                    sp = spsum.tile([P, IM], FP, name="sp", tag="sp")
                    for s in range(IM // 512):
                        nc.tensor.matmul(
                            sp[:, s * 512:(s + 1) * 512],
                            kt[po:po + DH, j * P:(j + 1) * P],
                            qt[po:po + DH,
                               imac * IM + s * 512:imac * IM + (s + 1) * 512],
                            start=True,
                            stop=True,
                        )
                    pt = ptp.tile([P, IM], BF, name="pt", tag="pt")
                    nc.scalar.activation(pt, sp, EXP, scale=SCALE)
                    for ic in range(NIC):
                        nc.tensor.matmul(
                            at[:, ic // 4, (ic % 4) * 65:(ic % 4) * 65 + 65],
                            pt[:, ic * P:(ic + 1) * P],
                            V[j][:, h, :],
                            start=(j == 0 and ic % 4 == 0),
                            stop=(j == NI - 1 and ic % 4 == 3),
                            skip_group_check=True,
                        )
                for ic in range(NIC):
                    blk = at[:, ic // 4, (ic % 4) * 65:(ic % 4) * 65 + 65]
                    rec = recp.tile([P, 1], FP, name="rec", tag="rec")
                    nc.vector.reciprocal(rec, blk[:, DH:DH + 1])
                    nc.vector.tensor_scalar_mul(
                        outs[ic][:, h * DH:(h + 1) * DH], blk[:, 0:DH], rec
                    )
                if h in (3, NH - 1):
                    # stream finished column halves out so the final DMA
                    # isn't serialized behind the last head's pipeline
                    c0 = 0 if h == 3 else CC // 2
                    for blk in range(NIC):
                        i0 = imac * IM + blk * P
                        nc.sync.dma_start(
                            out=out_d[i0:i0 + P, c0:c0 + CC // 2],
                            in_=outs[blk][:, c0:c0 + CC // 2],
                        )


def _build():
    global _NC
    if _NC is not None:
        return _NC
    nc = bacc.Bacc(None, target_bir_lowering=False, debug=False)
    with TileContext(nc) as tc:
        with tc.tile_pool(name="dram", bufs=1, space="DRAM") as dram:
            x_d = dram.tile([SEQ, DIM], BF, kind="ExternalInput", name="x",
                            uniquify=False)
            c_d = dram.tile([SEQ, DIM], BF, kind="ExternalInput", name="ctx",
                            uniquify=False)
            wq_d = dram.tile([DIM, CC], BF, kind="ExternalInput", name="wq",
                             uniquify=False)
            wk_d = dram.tile([DIM, CC], BF, kind="ExternalInput", name="wk",
                             uniquify=False)
            wv_d = dram.tile([DIM, CC], BF, kind="ExternalInput", name="wv",
                             uniquify=False)
            out_d = dram.tile([SEQ, CC], FP, kind="ExternalOutput", name="out",
                              uniquify=False)
            _build_body(nc, tc, x_d, c_d, wq_d, wk_d, wv_d, out_d)
    nc.compile()
    _NC = nc
    return nc


def make_in_maps(x, context, Wq, Wkv):
    bf16 = ml_dtypes.bfloat16
    x = np.asarray(x, dtype=np.float32).astype(bf16)
    context = np.asarray(context, dtype=np.float32).astype(bf16)
    Wq = np.asarray(Wq, dtype=np.float32).astype(bf16)
    Wkv = np.asarray(Wkv, dtype=np.float32).astype(bf16)
    in_maps = []
    for core in range(8):
        b, hg = divmod(core, 2)
        c0 = hg * CC
        in_maps.append({
            "x": np.ascontiguousarray(x[b]),
            "ctx": np.ascontiguousarray(context[b]),
            "wq": np.ascontiguousarray(Wq[:, c0:c0 + CC]),
            "wk": np.ascontiguousarray(Wkv[:, c0:c0 + CC]),
            "wv": np.ascontiguousarray(Wkv[:, DIM + c0:DIM + c0 + CC]),
        })
    return in_maps


def run(x, context, Wq, Wkv, **run_kwargs):
    nc = _build()
    in_maps = make_in_maps(x, context, Wq, Wkv)
    res = run_bass_kernel_spmd(nc, in_maps, core_ids=list(range(8)), **run_kwargs)
    out = np.empty((4, SEQ, DIM), dtype=np.float32)
    for core in range(8):
        b, hg = divmod(core, 2)
        out[b, :, hg * CC:(hg + 1) * CC] = res.results[core]["out"]
    return out, res


def kernel(x, context, Wq, Wkv):
    out, _ = run(x, context, Wq, Wkv)
    return out


# revision 18
# speedup vs baseline: 1.1558x; 1.1558x over previous
"""Cross-attention kernel for 8 Trainium2 NeuronCores.

Contract: kernel(**inputs) takes FULL unsharded numpy inputs
(x [4,2048,1024], context [4,2048,1024], Wq [1024,1024], Wkv [1024,2048])
and returns the full output [4, 2048, 1024] (float32).

Sharding (hardcoded): core = b * 2 + hg handles batch b (0..3) and head
group hg (0..1) = heads hg*8 .. hg*8+7 (16 heads total, d=64). Data +
tensor parallel: no cross-core communication (softmax is per-row).

Matmuls run in bf16 (fp32 is 2-pass LOW_HIGH on the PE = half
throughput); accumulation is fp32 in PSUM. Inputs are cast to bf16 on
the host. Output is fp32.

Per-core dataflow:
  cT = context[b].T               (PE transpose, bf16)
  KT = Wk_slice.T @ cT            [512 c, 2048 j] bf16
  V  = cT.T @ Wv_slice            [2048 j, 8 h, 65] bf16 (col 64 = 1.0)
  xT = x[b].T ; QT = Wq_slice.T @ xT   [512 c, 2048 i] bf16
  per (head h, i-macro of 1024):
    for j-chunk of 128:
      S^T = K_h^T' Q_h^T          [128 j, 1024 i] PSUM f32 (K=64 matmul)
      P^T = exp(S^T / 8)          ACT, PSUM -> SBUF bf16 (no max-sub:
                                   scores ~ N(0,1), exp is range-safe)
      per i-chunk of 128 (8):     natural-form attention accumulate
        at[:, ic] += P^T[:, ic].T @ [V_h|1]    [128 i, 65] PSUM f32
                                   (8 accumulators packed into 2 banks;
                                    start=True clears a whole bank, so
                                    only the first group per bank sets it)
    out_sb[:, h*64:+64] = at[..:64] * recip(at[.., 64])   (DVE, per ic)
  DMA out_sb -> out[2048, 512] f32 DRAM (host scatters into full out)

The attention inner loop is gated by ScalarE (exp); to keep the PE's
HAM governor warm (K=8), half the xT transposes, KT[1..3], and all QT
projection chunks are emitted as a metered filler queue between heads,
giving the scheduler dependency-free PE work for every bubble.
"""

import sys

if "/opt/trn_rl_repo" not in sys.path:
    sys.path.insert(0, "/opt/trn_rl_repo")

from contextlib import ExitStack

import ml_dtypes
import numpy as np

import concourse.bass as bass  # noqa: F401  (registers AP machinery)
import concourse.mybir as mybir
from concourse import bacc
from concourse.bass_utils import run_bass_kernel_spmd
from concourse.masks import make_identity
from concourse.tile import TileContext

FP = mybir.dt.float32
BF = mybir.dt.bfloat16
P = 128
SEQ = 2048
DIM = 1024
CC = 512  # per-core channel cols (8 heads x 64)
NH = 8  # heads per core
DH = 64  # head dim
NI = SEQ // P  # 16 seq chunks
NK = DIM // P  # 8 contraction chunks
IM = 1024  # i-macro width for attention
NIM = SEQ // IM  # 2
NIC = IM // P  # 8 i-chunks per macro
SCALE = DH ** -0.5

EXP = mybir.ActivationFunctionType.Exp

_NC = None


def _build_body(nc, tc, x_d, c_d, wq_d, wk_d, wv_d, out_d):
    with ExitStack() as ctx:
        const = ctx.enter_context(tc.tile_pool(name="const", bufs=1))
        ident = const.tile([P, P], BF, name="ident")
        make_identity(nc, ident)

        ctp = ctx.enter_context(tc.tile_pool(name="ctp", bufs=1))
        xtp = ctx.enter_context(tc.tile_pool(name="xtp", bufs=1))
        ktp = ctx.enter_context(tc.tile_pool(name="ktp", bufs=4))
        qtp = ctx.enter_context(tc.tile_pool(name="qtp", bufs=4))
        vp = ctx.enter_context(tc.tile_pool(name="vp", bufs=NI))
        wp = ctx.enter_context(tc.tile_pool(name="wp", bufs=24))
        natp = ctx.enter_context(tc.tile_pool(name="natp", bufs=4))
        ptp = ctx.enter_context(tc.tile_pool(name="ptp", bufs=4))
        outp = ctx.enter_context(tc.tile_pool(name="outp", bufs=10))
        recp = ctx.enter_context(tc.tile_pool(name="recp", bufs=8))
        # PSUM budget (8 banks): sp 2x2 + at 1x2 + fill 2x1 = 8
        fillp = ctx.enter_context(tc.tile_pool(name="fillp", bufs=2, space="PSUM"))
        spsum = ctx.enter_context(tc.tile_pool(name="spsum", bufs=2, space="PSUM"))
        apsum = ctx.enter_context(tc.tile_pool(name="apsum", bufs=1, space="PSUM"))

        KT = [ktp.tile([P, SEQ], BF, name=f"kt{m}", tag="kt") for m in range(4)]
        QT = [qtp.tile([P, SEQ], BF, name=f"qt{m}", tag="qt") for m in range(4)]
        V = [vp.tile([P, NH, DH + 1], BF, name=f"v{j}", tag="v") for j in range(NI)]
        # consolidated transposed activations: [:, k, :] is the k-th
        # 128-row contraction slice (lets 4 transposes share one eviction)
        cT = ctp.tile([P, NK, SEQ], BF, name="ct", tag="act")
        xT = xtp.tile([P, NK, SEQ], BF, name="xt", tag="act2")

        def transpose_chunk(dst, src_d, i):
            # one [128, 1024] row block of src -> dst[:, :, i*128:+128];
            # 4 transposes share a PSUM bank (only the first may set
            # start: start=True clears the whole bank) and one eviction.
            nat = natp.tile([P, DIM], BF, name="nat", tag="nat")
            nc.sync.dma_start(out=nat, in_=src_d[i * P:(i + 1) * P, :])
            for half in range(2):
                tp = fillp.tile([P, 512], BF, name="tp", tag="fp")
                for q in range(4):
                    k = half * 4 + q
                    nc.tensor.matmul(
                        tp[:, q * P:(q + 1) * P],
                        nat[:, k * P:(k + 1) * P],
                        ident,
                        is_transpose=True,
                        start=(q == 0),
                        stop=(q == 3),
                        skip_group_check=True,
                    )
                nc.vector.tensor_copy(
                    dst[:, half * 4:half * 4 + 4, i * P:(i + 1) * P],
                    tp.rearrange("p (k c) -> p k c", k=4),
                )

        def proj_chunk(dst, w, src, m, i4):
            # dst[m][:, i4*512:+512] = sum_k w[k][:, m-slice].T @ src[:, k, i4]
            ps = fillp.tile([P, 512], FP, name="ps", tag="fp")
            for k in range(NK):
                nc.tensor.matmul(
                    ps,
                    w[k][:, m * P:(m + 1) * P],
                    src[:, k, i4 * 512:(i4 + 1) * 512],
                    start=(k == 0),
                    stop=(k == NK - 1),
                )
            nc.vector.tensor_copy(dst[m][:, i4 * 512:(i4 + 1) * 512], ps)

        def v_chunk(j):
            ps = fillp.tile([P, 512], FP, name="psv", tag="fp")
            for k in range(NK):
                nc.tensor.matmul(
                    ps,
                    cT[:, k, j * P:(j + 1) * P],
                    wv[k],
                    start=(k == 0),
                    stop=(k == NK - 1),
                )
            nc.vector.tensor_copy(
                V[j][:, :, 0:DH], ps.rearrange("p (h d) -> p h d", h=NH)
            )
            nc.vector.memset(V[j][:, :, DH:DH + 1], 1.0)

        # ---- minimal serial prefix ----
        for i in range(4):
            transpose_chunk(cT, c_d, i)
        wk = [wp.tile([P, CC], BF, name=f"wk{k}", tag="w") for k in range(NK)]
        wv = [wp.tile([P, CC], BF, name=f"wv{k}", tag="w") for k in range(NK)]
        wq = [wp.tile([P, CC], BF, name=f"wq{k}", tag="w") for k in range(NK)]
        for k in range(NK):
            nc.sync.dma_start(out=wk[k], in_=wk_d[k * P:(k + 1) * P, :])
            nc.sync.dma_start(out=wv[k], in_=wv_d[k * P:(k + 1) * P, :])
            nc.sync.dma_start(out=wq[k], in_=wq_d[k * P:(k + 1) * P, :])
        proj_chunk(KT, wk, cT, 0, 0)
        for j in range(4):
            v_chunk(j)
        for i in range(NIC):
            transpose_chunk(xT, x_d, i)
        proj_chunk(QT, wq, xT, 0, 0)
        proj_chunk(QT, wq, xT, 0, 1)

        # ---- j-granular filler: everything else streams through the
        # attention phase so the PE never drains (deadlines honored).
        def ct_u(i):
            return lambda: transpose_chunk(cT, c_d, i)

        def xt_u(i):
            return lambda: transpose_chunk(xT, x_d, i)

        def kt_u(m, i4):
            return lambda: proj_chunk(KT, wk, cT, m, i4)

        def qt_u(m, i4):
            return lambda: proj_chunk(QT, wq, xT, m, i4)

        def v_u(j):
            return lambda: v_chunk(j)

        filler = {
            (0, 0, 0): [ct_u(4), ct_u(5)],
            (0, 0, 1): [ct_u(6), ct_u(7)],
            (0, 0, 2): [kt_u(0, 1), v_u(4)],
            (0, 0, 3): [ct_u(8), v_u(5)],
            (0, 0, 4): [ct_u(9), v_u(6)],
            (0, 0, 5): [ct_u(10), v_u(7)],
            (0, 0, 6): [ct_u(11), kt_u(0, 2), v_u(8)],
            (0, 0, 7): [ct_u(12), v_u(9)],
            (0, 0, 8): [ct_u(13), v_u(10)],
            (0, 0, 9): [ct_u(14), v_u(11)],
            (0, 0, 10): [ct_u(15), kt_u(0, 3), v_u(12)],
            (0, 0, 11): [v_u(13)],
            (0, 0, 12): [v_u(14)],
            (0, 0, 13): [v_u(15)],
            (0, 1, 0): [kt_u(1, 0)], (0, 1, 2): [kt_u(1, 1)],
            (0, 1, 4): [kt_u(1, 2)], (0, 1, 6): [kt_u(1, 3)],
            (0, 1, 8): [qt_u(1, 0)], (0, 1, 11): [qt_u(1, 1)],
            (0, 2, 0): [kt_u(2, 0)], (0, 2, 4): [kt_u(2, 1)],
            (0, 2, 8): [kt_u(2, 2)], (0, 2, 12): [kt_u(2, 3)],
            (0, 3, 0): [qt_u(2, 0)], (0, 3, 8): [qt_u(2, 1)],
            (0, 4, 0): [kt_u(3, 0)], (0, 4, 4): [kt_u(3, 1)],
            (0, 4, 8): [kt_u(3, 2)], (0, 4, 12): [kt_u(3, 3)],
            (0, 5, 0): [qt_u(3, 0)], (0, 5, 8): [qt_u(3, 1)],
            (0, 6, 0): [xt_u(8)], (0, 6, 2): [xt_u(9)],
            (0, 6, 4): [xt_u(10)], (0, 6, 6): [xt_u(11)],
            (0, 6, 8): [xt_u(12)], (0, 6, 10): [xt_u(13)],
            (0, 6, 12): [xt_u(14)], (0, 6, 14): [xt_u(15)],
            (0, 7, 0): [qt_u(0, 2)], (0, 7, 8): [qt_u(0, 3)],
            (1, 0, 0): [qt_u(1, 2)], (1, 0, 8): [qt_u(1, 3)],
            (1, 2, 0): [qt_u(2, 2)], (1, 2, 8): [qt_u(2, 3)],
            (1, 4, 0): [qt_u(3, 2)], (1, 4, 8): [qt_u(3, 3)],
        }

        # ---------------- attention ----------------
        for imac in range(NIM):
            outs = [
                outp.tile([P, CC], FP, name=f"o{imac}_{b}", tag="o")
                for b in range(NIC)
            ]
            for h in range(NH):
                m = h // 2
                kt = KT[m]
                qt = QT[m]
                po = (h % 2) * DH
                at = apsum.tile([P, 2, 512], FP, name="at", tag="at")
                for j in range(NI):
                    units = filler.get((imac, h, j), ())
                    for thunk in units:
                        thunk()
                    sp = spsum.tile([P, IM], FP, name="sp", tag="sp")
                    for s in range(IM // 512):
                        nc.tensor.matmul(
                            sp[:, s * 512:(s + 1) * 512],
                            kt[po:po + DH, j * P:(j + 1) * P],
                            qt[po:po + DH,
                               imac * IM + s * 512:imac * IM + (s + 1) * 512],
                            start=True,
                            stop=True,
                        )
                    pt = ptp.tile([P, IM], BF, name="pt", tag="pt")
                    nc.scalar.activation(pt, sp, EXP, scale=SCALE)
                    for ic in range(NIC):
                        nc.tensor.matmul(
                            at[:, ic // 4, (ic % 4) * 65:(ic % 4) * 65 + 65],
                            pt[:, ic * P:(ic + 1) * P],
                            V[j][:, h, :],
                            start=(j == 0 and ic % 4 == 0),
                            stop=(j == NI - 1 and ic % 4 == 3),
                            skip_group_check=True,
                        )
                for ic in range(NIC):
                    blk = at[:, ic // 4, (ic % 4) * 65:(ic % 4) * 65 + 65]
                    rec = recp.tile([P, 1], FP, name="rec", tag="rec")
                    nc.vector.reciprocal(rec, blk[:, DH:DH + 1])
                    nc.vector.tensor_scalar_mul(
                        outs[ic][:, h * DH:(h + 1) * DH], blk[:, 0:DH], rec
                    )
            for blk in range(NIC):
                i0 = imac * IM + blk * P
                nc.sync.dma_start(out=out_d[i0:i0 + P, :], in_=outs[blk])


def _build():
    global _NC
    if _NC is not None:
        return _NC
    nc = bacc.Bacc(None, target_bir_lowering=False, debug=False)
    with TileContext(nc) as tc:
        with tc.tile_pool(name="dram", bufs=1, space="DRAM") as dram:
            x_d = dram.tile([SEQ, DIM], BF, kind="ExternalInput", name="x",
                            uniquify=False)
            c_d = dram.tile([SEQ, DIM], BF, kind="ExternalInput", name="ctx",
                            uniquify=False)
            wq_d = dram.tile([DIM, CC], BF, kind="ExternalInput", name="wq",
                             uniquify=False)
            wk_d = dram.tile([DIM, CC], BF, kind="ExternalInput", name="wk",
                             uniquify=False)
            wv_d = dram.tile([DIM, CC], BF, kind="ExternalInput", name="wv",
                             uniquify=False)
            out_d = dram.tile([SEQ, CC], FP, kind="ExternalOutput", name="out",
                              uniquify=False)
            _build_body(nc, tc, x_d, c_d, wq_d, wk_d, wv_d, out_d)
    nc.compile()
    _NC = nc
    return nc


def make_in_maps(x, context, Wq, Wkv):
    bf16 = ml_dtypes.bfloat16
    x = np.asarray(x, dtype=np.float32).astype(bf16)
    context = np.asarray(context, dtype=np.float32).astype(bf16)
    Wq = np.asarray(Wq, dtype=np.float32).astype(bf16)
    Wkv = np.asarray(Wkv, dtype=np.float32).astype(bf16)
    in_maps = []
    for core in range(8):
        b, hg = divmod(core, 2)
        c0 = hg * CC
        in_maps.append({
            "x": np.ascontiguousarray(x[b]),
            "ctx": np.ascontiguousarray(context[b]),
            "wq": np.ascontiguousarray(Wq[:, c0:c0 + CC]),
            "wk": np.ascontiguousarray(Wkv[:, c0:c0 + CC]),
            "wv": np.ascontiguousarray(Wkv[:, DIM + c0:DIM + c0 + CC]),
        })
    return in_maps


def run(x, context, Wq, Wkv, **run_kwargs):
    nc = _build()
    in_maps = make_in_maps(x, context, Wq, Wkv)
    res = run_bass_kernel_spmd(nc, in_maps, core_ids=list(range(8)), **run_kwargs)
    out = np.empty((4, SEQ, DIM), dtype=np.float32)
    for core in range(8):
        b, hg = divmod(core, 2)
        out[b, :, hg * CC:(hg + 1) * CC] = res.results[core]["out"]
    return out, res


def kernel(x, context, Wq, Wkv):
    out, _ = run(x, context, Wq, Wkv)
    return out
